# revision 47
# baseline (speedup 1.0000x reference)
"""Trainium2 Bass kernel for nn_MultiHeadAttention (decode-style, q_len=1).

Data-parallel over batch: 64 batches -> 8 cores x 8 batches.

Algebraic restructuring (exact, exploits q_len == 1):
  scores[b,h,s] = k[b,s,:] . R_b[:,h] + const(b,h)   # const drops in softmax
     where R_b[d,h] = sum_{d'} Wk[d, h*64+d'] qh[b, h*64+d']
  out_concat[b,hd] = (sum_s p[b,h,s] v[b,s,:]) @ Wv[:,hd] + bv[hd]
so the big K/V projections are never computed; k and v are contracted
directly and the kernel is HBM-bound on streaming k,v.

Precision/layout staging (host side, per-core):
  k  -> fp8 e3m4, pre-transposed [128(d%), 8(d/128), S]   (16 MiB/core)
  v  -> bf16,     chunked       [128(s%), S/128, 1024]    (32 MiB/core)
  Wq/Wk^T/Wv/Wo -> bf16 pre-transposed SBUF layouts       ( 8 MiB/core)
Total ~56 MiB/core HBM reads (vs 144 f32), no on-device transposes or
casts of the streams.  Numpy-simulated rel err 1.41e-2 (< 2e-2 gate);
the e3m4 path keeps 4 mantissa bits and max 15.5 (|k| <= ~5.5).

Schedule: THREE DMA queues (sync=SP HWDGE, scalar=Act HWDGE,
gpsimd=SWDGE) stream in parallel; each batch's three pieces (kt, v-lo,
v-hi) rotate across the queues so arrival order tracks consumption
order and all queues carry ~19-21 MB; Wq/Wk^T head the queues as 2KB
column chunks the R-chain chases; Wv/Wo slot in mid-stream.  The batch
loop is software-pipelined for the in-order PE queue: scores(b) [32
matmuls, bf16 R x fp8 k] issue before U(b-1) [32 matmuls, bf16 ET x
bf16 v], so a late v can never head-block the next batch's scores, and
exp(b) (scalar engine, fused den accumulation) overlaps U(b-1).  The
out-projection computes all (h,b) x Wv products in 8 wide matmuls and
extracts the valid block-diagonal in transposed (OCT) layout during
the PSUM->SBUF copy (fused with bv), then chases y = relu(OC@Wo + bo)
per column chunk, one chunk behind the extraction.
"""

import numpy as np
import ml_dtypes
from contextlib import ExitStack

import concourse.bass as bass
import concourse.tile as tile
from concourse import bacc, mybir
from concourse.bass_utils import run_bass_kernel_spmd

try:
    import axon_profile_shim
    axon_profile_shim.install()
except Exception:
    pass

N_CORES = 8
D = 1024
H = 16
DK = 64
F32 = mybir.dt.float32
BF16 = mybir.dt.bfloat16
FP8 = mybir.dt.float8e3
AX = mybir.AxisListType
ALU = mybir.AluOpType
ACTF = mybir.ActivationFunctionType

NP_BF16 = ml_dtypes.bfloat16
NP_FP8 = ml_dtypes.float8_e3m4


def _make_identity(nc, ap):
    nc.gpsimd.memset(ap, 0.0)
    nc.gpsimd.affine_select(
        out=ap, in_=ap, compare_op=ALU.not_equal, fill=1.0,
        base=0, pattern=[[-1, ap.shape[0]]], channel_multiplier=1,
    )


def build(BL=8, S=2048, n_cores=N_CORES):
    """Build + compile the per-core program. BL = local batches, S = seq len."""
    SC = S // 128           # 128-row s-chunks
    SG = S // 512           # 512-col score blocks
    HB = H * BL
    nc = bacc.Bacc("TRN2", target_bir_lowering=False, debug=False,
                   num_devices=n_cores)

    kt_ext = nc.dram_tensor("kt", [BL, 128, 8, S], FP8, kind="ExternalInput").ap()
    vt_ext = nc.dram_tensor("vt", [BL, 128, SC, D], BF16, kind="ExternalInput").ap()
    qt_ext = nc.dram_tensor("qt", [128, 8, BL], BF16, kind="ExternalInput").ap()
    wq_ext = nc.dram_tensor("wq", [128, 8, D], BF16, kind="ExternalInput").ap()
    wkt_ext = nc.dram_tensor("wkt", [128, 8, D], BF16, kind="ExternalInput").ap()
    wv_ext = nc.dram_tensor("wv", [128, 8, D], BF16, kind="ExternalInput").ap()
    wo_ext = nc.dram_tensor("wo", [128, 8, D], BF16, kind="ExternalInput").ap()
    bq8_ext = nc.dram_tensor("bq8", [BL, D], F32, kind="ExternalInput").ap()
    bvt_ext = nc.dram_tensor("bvt", [128, 8], F32, kind="ExternalInput").ap()
    bo8_ext = nc.dram_tensor("bo8", [BL, D], F32, kind="ExternalInput").ap()
    y_ext = nc.dram_tensor("y", [BL, D], F32, kind="ExternalOutput").ap()

    with tile.TileContext(nc) as tc, ExitStack() as ctx:
        cpool = ctx.enter_context(tc.tile_pool(name="const", bufs=1))
        ident = cpool.tile([128, 128], F32)
        _make_identity(nc, ident[:])
        qt_sb = cpool.tile([128, 8, BL], BF16)
        nc.sync.dma_start(qt_sb[:], qt_ext[:])
        bvt = cpool.tile([128, 8], F32)
        nc.sync.dma_start(bvt[:], bvt_ext[:])

        # persistent across whole kernel
        R_all = cpool.tile([128, 8, HB], BF16)
        UT_all = cpool.tile([128, 8, H, BL], BF16)

        # ---------------- setup: qh^T, R ----------------
        with tc.tile_pool(name="wsetup", bufs=1) as wpool, \
             tc.tile_pool(name="spsum", bufs=1, space="PSUM") as spsum:
            bq8 = wpool.tile([BL, D], F32)
            nc.sync.dma_start(bq8[:], bq8_ext[:])
            # chunked weight loads spread over all 3 queues so the 4.2 MB
            # R-chain input lands in ~1/3 the time; R compute chases chunks
            QS0 = [nc.sync, nc.scalar, nc.gpsimd]
            wq_sb = wpool.tile([128, 8, D], BF16)
            wkt_sb = wpool.tile([128, 8, D], BF16)
            for cch in range(8):
                QS0[cch % 3].dma_start(wq_sb[:, cch, :], wq_ext[:, cch, :])
                QS0[(cch + 1) % 3].dma_start(wkt_sb[:, cch, :], wkt_ext[:, cch, :])

            # qh = q @ Wq + bq   [BL, D]
            qhp = spsum.tile([BL, D], F32, tag="qhp")
            for i in range(8):
                for n in range(2):
                    nc.tensor.matmul(qhp[:, n * 512:(n + 1) * 512],
                                     qt_sb[:, i, :],
                                     wq_sb[:, i, n * 512:(n + 1) * 512],
                                     start=(i == 0), stop=(i == 7))
            qh_sb = wpool.tile([BL, D], F32)
            nc.vector.tensor_add(qh_sb[:], qhp[:], bq8[:])
            qtp = spsum.tile([128, 8 * BL], F32, tag="qtp")
            for m in range(8):
                nc.tensor.transpose(qtp[:, m * BL:(m + 1) * BL],
                                    qh_sb[:, m * 128:(m + 1) * 128],
                                    ident[:BL, :BL])
            qhT_sb = wpool.tile([128, 8 * BL], F32)   # [p, m*BL + b]
            nc.vector.tensor_copy(qhT_sb[:], qtp[:])

            # Block-diagonal qh (bf16) for ALL batches:
            # qblk_c[p, b*16+h] = qh_b[c*128+p] if h == head(c*128+p) else 0
            qblk = [wpool.tile([128, HB], BF16, tag=f"qblk{c}", name=f"qblk{c}")
                    for c in range(8)]
            for c in range(8):
                nc.vector.memset(qblk[c][:], 0.0)
                lo = qblk[c][0:64, :].rearrange("p (b h) -> p b h", h=H)
                hi = qblk[c][64:128, :].rearrange("p (b h) -> p b h", h=H)
                nc.vector.tensor_copy(
                    lo[:, :, 2 * c:2 * c + 1],
                    qhT_sb[0:64, c * BL:(c + 1) * BL].unsqueeze(2))
                nc.vector.tensor_copy(
                    hi[:, :, 2 * c + 1:2 * c + 2],
                    qhT_sb[64:128, c * BL:(c + 1) * BL].unsqueeze(2))

            # RT[(b,h), d] = sum_c qblk_c^T @ WkT_c
            rtp = [spsum.tile([HB, 512], F32, tag=f"rtp{n}", name=f"rtp{n}")
                   for n in range(2)]
            for c in range(8):
                for n in range(2):
                    nc.tensor.matmul(rtp[n][:], qblk[c][:],
                                     wkt_sb[:, c, n * 512:(n + 1) * 512],
                                     start=(c == 0), stop=(c == 7))
            RT_sb = wpool.tile([HB, D], F32)
            for n in range(2):
                nc.vector.tensor_copy(RT_sb[:, n * 512:(n + 1) * 512], rtp[n][:])
            for j in range(8):
                rp = spsum.tile([128, HB], F32, tag="rp", name="rp")
                nc.tensor.transpose(rp[:], RT_sb[:, j * 128:(j + 1) * 128],
                                    ident[:HB, :HB])
                nc.vector.tensor_copy(R_all[:, j, :], rp[:])

        # ---------------- stream pools (reuse setup SBUF) ----------------
        tailw = ctx.enter_context(tc.tile_pool(name="tailw", bufs=1))
        wv_sb = tailw.tile([128, 8, D], BF16)
        wo_sb = tailw.tile([128, 8, D], BF16)
        bo8 = tailw.tile([BL, D], F32)

        stream_sbuf = ExitStack()
        epool = stream_sbuf.enter_context(tc.tile_pool(name="epool", bufs=2))
        etpool = stream_sbuf.enter_context(tc.tile_pool(name="etpool", bufs=2))
        upool = stream_sbuf.enter_context(tc.tile_pool(name="upool", bufs=2))
        ktpool = stream_sbuf.enter_context(tc.tile_pool(name="ktpool", bufs=3))
        vpool = stream_sbuf.enter_context(tc.tile_pool(name="vpool", bufs=3))

        stream_psum = ExitStack()
        scp = stream_psum.enter_context(tc.tile_pool(name="scp", bufs=2, space="PSUM"))
        upp = stream_psum.enter_context(tc.tile_pool(name="upp", bufs=1, space="PSUM"))
        tpp = stream_psum.enter_context(tc.tile_pool(name="tpp", bufs=1, space="PSUM"))
        tp2 = stream_psum.enter_context(tc.tile_pool(name="tp2", bufs=1, space="PSUM"))

        # rotate each batch's three pieces (kt, v-lower, v-upper) across the
        # three DMA queues so arrival order tracks consumption order and all
        # queues carry ~19-21 MB.  kt is issued 3 batches ahead (bufs=4, it
        # gates scores); v only 1 ahead so its buffer-wait (bufs=2) never
        # blocks a queue head.
        QS = [nc.sync, nc.scalar, nc.gpsimd]

        def load_kt(b):
            kt_t = ktpool.tile([128, 8, S], FP8, tag="kt", name="kt")
            QS[b % 3].dma_start(kt_t[:], kt_ext[b])
            return kt_t

        def load_v(b):
            vf = vpool.tile([128, SC, D], BF16, tag="vf", name="vf")
            half = SC // 2
            QS[(b + 1) % 3].dma_start(vf[:, :half, :], vt_ext[b, :, :half, :])
            QS[(b + 2) % 3].dma_start(vf[:, half:, :], vt_ext[b, :, half:, :])
            return vf

        kts = [load_kt(0)]
        vfs = [load_v(0)]
        if BL > 1:
            kts.append(load_kt(1))

        # ---------------- stream phase (software-pipelined) ----------------
        # PE issue order per iteration: scores(b) then U(b-1) — so a late
        # v(b-1) can never head-block scores(b) in the in-order PE queue,
        # and the PE gets 64 back-to-back big matmuls per iteration (stays
        # at full DVFS ramp).
        SGG = max(SG // 2, 1)       # [H, 1024] score blocks
        W2 = min(S, 1024)

        def scores_phase(b, kt_t):
            den4 = epool.tile([H, SGG], F32, tag="den4")
            scs = []
            for g in range(SGG):
                sc = scp.tile([H, W2], F32, tag="sc")
                for j in range(8):
                    for n in range(W2 // 512):
                        nc.tensor.matmul(
                            sc[:, n * 512:(n + 1) * 512],
                            R_all[:, j, b * H:(b + 1) * H],
                            kt_t[:, j, g * W2 + n * 512:g * W2 + (n + 1) * 512],
                            start=(j == 0), stop=(j == 7))
                scs.append(sc)
            return den4, scs

        def exp_trans_phase(b, den4, scs):
            Es = []
            for g, sc in enumerate(scs):
                E_g = epool.tile([H, W2], F32, tag="E")
                nc.scalar.activation(E_g[:], sc[:], ACTF.Exp, scale=0.125,
                                     accum_out=den4[:, g:g + 1])
                Es.append(E_g)
            sp = tpp.tile([128, SC * H], F32, tag="sp")
            for g, E_g in enumerate(Es):
                for i in range(W2 // 128):
                    t = g * (W2 // 128) + i
                    nc.tensor.transpose(sp[:, t * H:(t + 1) * H],
                                        E_g[:, i * 128:(i + 1) * 128],
                                        ident[:H, :H])
            ET = etpool.tile([128, SC, H], BF16, tag="ET")
            nc.vector.tensor_copy(
                ET[:], sp[:].rearrange("p (t h) -> p t h", t=SC))
            return ET

        def u_mm_phase(ET, vf):
            up = upp.tile([H, D], F32, tag="up")
            for cc in range(SC):
                for n in range(2):
                    nc.tensor.matmul(up[:, n * 512:(n + 1) * 512],
                                     ET[:, cc, :],
                                     vf[:, cc, n * 512:(n + 1) * 512],
                                     start=(cc == 0), stop=(cc == SC - 1))
            return up

        def u_finish_phase(b, den4, up):
            den = epool.tile([H, 1], F32, tag="den")
            nc.vector.tensor_reduce(den[:], den4[:], axis=AX.X, op=ALU.add)
            rden = epool.tile([H, 1], F32, tag="rden")
            nc.vector.reciprocal(rden[:], den[:])
            U_sb = upool.tile([H, D], F32, tag="U")
            nc.vector.tensor_scalar_mul(U_sb[:], up[:], rden[:])
            sp2 = tp2.tile([128, 8 * H], F32, tag="sp2")
            for jc in range(8):
                nc.tensor.transpose(sp2[:, jc * H:(jc + 1) * H],
                                    U_sb[:, jc * 128:(jc + 1) * 128],
                                    ident[:H, :H])
            nc.vector.tensor_copy(
                UT_all[:, :, :, b],
                sp2[:].rearrange("p (j h) -> p j h", j=8))

        prev = None     # (b, den4, ET)
        for b in range(BL):
            if b + 1 < BL:
                vfs.append(load_v(b + 1))
            if b + 2 < BL:
                kts.append(load_kt(b + 2))
            if b == min(4, BL - 1):
                nc.sync.dma_start(wv_sb[:], wv_ext[:])
                nc.scalar.dma_start(wo_sb[:], wo_ext[:])
                nc.sync.dma_start(bo8[:], bo8_ext[:])


            den4, scs = scores_phase(b, kts[b])
            if prev is not None:
                pb, pden4, pET = prev
                pup = u_mm_phase(pET, vfs[pb])
            ET = exp_trans_phase(b, den4, scs)
            if prev is not None:
                u_finish_phase(pb, pden4, pup)
            prev = (b, den4, ET)

        pb, pden4, pET = prev
        pup = u_mm_phase(pET, vfs[pb])
        u_finish_phase(pb, pden4, pup)

        # ---------------- tail: out-projection ----------------
        # ocT[col, (h,b)] = sum_d Wv[d, col] U[b,h,d] for ALL (col, h) pairs;
        # valid block-diagonal (h == col//64) extracted during the PSUM->SBUF
        # copy, directly in transposed (OCT) layout for y = OC@Wo.
        stream_psum.close()
        stream_sbuf.close()
        with tc.tile_pool(name="fin", bufs=1) as fpool, \
             tc.tile_pool(name="fpsum", bufs=2, space="PSUM") as fpsum:
            # pipelined: oct(c+1) matmuls overlap the DVE extraction of
            # oct(c); ypp(c) follows one chunk behind so the PE never waits
            OCT = fpool.tile([128, 8, BL], BF16)
            ypp = fpsum.tile([BL, D], F32, tag="yp")

            oct_all = fpsum.tile([128, 8, HB], F32, tag="oct", name="oct")

            def oct_chunk(c):
                for jc in range(8):
                    nc.tensor.matmul(oct_all[:, c, :],
                                     wv_sb[:, jc, c * 128:(c + 1) * 128],
                                     UT_all[:, jc, :, :],
                                     start=(jc == 0), stop=(jc == 7))
                for half in range(2):
                    h = 2 * c + half
                    sl = slice(half * 64, (half + 1) * 64)
                    nc.vector.tensor_scalar_add(
                        OCT[sl, c, :], oct_all[sl, c, h * BL:(h + 1) * BL],
                        bvt[sl, c:c + 1])

            def ypp_chunk(c):
                for n in range(2):
                    nc.tensor.matmul(ypp[:, n * 512:(n + 1) * 512],
                                     OCT[:, c, :],
                                     wo_sb[:, c, n * 512:(n + 1) * 512],
                                     start=(c == 0), stop=(c == 7))

            oct_chunk(0)
            for c in range(1, 8):
                oct_chunk(c)
                ypp_chunk(c - 1)
            ypp_chunk(7)
            ytmp = fpool.tile([BL, D], F32)
            nc.vector.tensor_add(ytmp[:], ypp[:], bo8[:])
            y_sb = fpool.tile([BL, D], F32)
            nc.vector.tensor_scalar_max(y_sb[:], ytmp[:], 0.0)
            nc.sync.dma_start(y_ext[:], y_sb[:])

    nc.compile()
    return nc


_built = {}


def _get_nc(BL, S):
    key = (BL, S)
    if key not in _built:
        _built[key] = build(BL, S)
    return _built[key]


def kernel(q, k, v, Wq, bq, Wk, bk, Wv, bv, Wo, bo, _trace=False):
    q = np.asarray(q, dtype=np.float32)
    k = np.asarray(k, dtype=np.float32)
    v = np.asarray(v, dtype=np.float32)
    B, S = k.shape[0], k.shape[1]
    BL = B // N_CORES
    SC = S // 128
    nc = _get_nc(BL, S)

    # host-side staging: dtype + layout only (all model math is on-device)
    kt_all = k.astype(NP_FP8).reshape(B, S, 8, 128).transpose(0, 3, 2, 1)
    vt_all = v.astype(NP_BF16).reshape(B, SC, 128, D).transpose(0, 2, 1, 3)
    qt_all = q.reshape(B, 8, 128).transpose(2, 1, 0).astype(NP_BF16)  # [128,8,B]

    shared = {
        "wq": np.ascontiguousarray(
            Wq.astype(NP_BF16).reshape(8, 128, D).transpose(1, 0, 2)),
        "wkt": np.ascontiguousarray(
            np.ascontiguousarray(Wk.T).astype(NP_BF16)
            .reshape(8, 128, D).transpose(1, 0, 2)),
        "wv": np.ascontiguousarray(
            Wv.astype(NP_BF16).reshape(8, 128, D).transpose(1, 0, 2)),
        "wo": np.ascontiguousarray(
            Wo.astype(NP_BF16).reshape(8, 128, D).transpose(1, 0, 2)),
        "bvt": np.ascontiguousarray(
            np.asarray(bv, dtype=np.float32).reshape(8, 128).T),
        "bo8": np.ascontiguousarray(np.broadcast_to(
            np.asarray(bo, dtype=np.float32), (BL, D))),
        "bq8": np.ascontiguousarray(np.broadcast_to(
            np.asarray(bq, dtype=np.float32), (BL, D))),
    }
    in_maps = []
    for c in range(N_CORES):
        sl = slice(c * BL, (c + 1) * BL)
        in_maps.append({
            "kt": np.ascontiguousarray(kt_all[sl]),
            "vt": np.ascontiguousarray(vt_all[sl]),
            "qt": np.ascontiguousarray(qt_all[:, :, sl]),
            **shared,
        })
    res = run_bass_kernel_spmd(nc, in_maps, list(range(N_CORES)), trace=_trace)
    out = np.concatenate([res.results[c]["y"] for c in range(N_CORES)], axis=0)
    if _trace:
        kernel._last_exec_time_ns = res.exec_time_ns
        kernel._last_profile = res.profile_json
    return out


# revision 48
# speedup vs baseline: 1.0161x; 1.0161x over previous
"""Trainium2 Bass kernel for nn_MultiHeadAttention (decode-style, q_len=1).

Data-parallel over batch: 64 batches -> 8 cores x 8 batches.

Algebraic restructuring (exact, exploits q_len == 1):
  scores[b,h,s] = k[b,s,:] . R_b[:,h] + const(b,h)   # const drops in softmax
     where R_b[d,h] = sum_{d'} Wk[d, h*64+d'] qh[b, h*64+d']
  out_concat[b,hd] = (sum_s p[b,h,s] v[b,s,:]) @ Wv[:,hd] + bv[hd]
so the big K/V projections are never computed; k and v are contracted
directly and the kernel is HBM-bound on streaming k,v.

Precision/layout staging (host side, per-core):
  k  -> fp8 e3m4, pre-transposed [128(d%), 8(d/128), S]   (16 MiB/core)
  v  -> bf16,     chunked       [128(s%), S/128, 1024]    (32 MiB/core)
  Wq/Wk^T/Wv/Wo -> bf16 pre-transposed SBUF layouts       ( 8 MiB/core)
Total ~56 MiB/core HBM reads (vs 144 f32), no on-device transposes or
casts of the streams.  Numpy-simulated rel err 1.41e-2 (< 2e-2 gate);
the e3m4 path keeps 4 mantissa bits and max 15.5 (|k| <= ~5.5).

Schedule: THREE DMA queues (sync=SP HWDGE, scalar=Act HWDGE,
gpsimd=SWDGE) stream in parallel; each batch's three pieces (kt, v-lo,
v-hi) rotate across the queues so arrival order tracks consumption
order and all queues carry ~19-21 MB; Wq/Wk^T head the queues as 2KB
column chunks the R-chain chases; Wv/Wo slot in mid-stream.  The batch
loop is software-pipelined for the in-order PE queue: scores(b) [32
matmuls, bf16 R x fp8 k] issue before U(b-1) [32 matmuls, bf16 ET x
bf16 v], so a late v can never head-block the next batch's scores, and
exp(b) (scalar engine, fused den accumulation) overlaps U(b-1).  The
out-projection computes all (h,b) x Wv products in 8 wide matmuls and
extracts the valid block-diagonal in transposed (OCT) layout during
the PSUM->SBUF copy (fused with bv), then chases y = relu(OC@Wo + bo)
per column chunk, one chunk behind the extraction.
"""

import numpy as np
import ml_dtypes
from contextlib import ExitStack

import concourse.bass as bass
import concourse.tile as tile
from concourse import bacc, mybir
from concourse.bass_utils import run_bass_kernel_spmd

try:
    import axon_profile_shim
    axon_profile_shim.install()
except Exception:
    pass

N_CORES = 8
D = 1024
H = 16
DK = 64
F32 = mybir.dt.float32
BF16 = mybir.dt.bfloat16
FP8 = mybir.dt.float8e3
AX = mybir.AxisListType
ALU = mybir.AluOpType
ACTF = mybir.ActivationFunctionType

NP_BF16 = ml_dtypes.bfloat16
NP_FP8 = ml_dtypes.float8_e3m4


def _make_identity(nc, ap):
    nc.gpsimd.memset(ap, 0.0)
    nc.gpsimd.affine_select(
        out=ap, in_=ap, compare_op=ALU.not_equal, fill=1.0,
        base=0, pattern=[[-1, ap.shape[0]]], channel_multiplier=1,
    )


def build(BL=8, S=2048, n_cores=N_CORES):
    """Build + compile the per-core program. BL = local batches, S = seq len."""
    SC = S // 128           # 128-row s-chunks
    SG = S // 512           # 512-col score blocks
    HB = H * BL
    nc = bacc.Bacc("TRN2", target_bir_lowering=False, debug=False,
                   num_devices=n_cores)

    kt_ext = nc.dram_tensor("kt", [BL, 128, 8, S], FP8, kind="ExternalInput").ap()
    vt_ext = nc.dram_tensor("vt", [BL, 128, SC, D], BF16, kind="ExternalInput").ap()
    qt_ext = nc.dram_tensor("qt", [128, 8, BL], BF16, kind="ExternalInput").ap()
    wq_ext = nc.dram_tensor("wq", [128, 8, D], BF16, kind="ExternalInput").ap()
    wkt_ext = nc.dram_tensor("wkt", [128, 8, D], BF16, kind="ExternalInput").ap()
    wv_ext = nc.dram_tensor("wv", [128, 8, D], BF16, kind="ExternalInput").ap()
    wo_ext = nc.dram_tensor("wo", [128, 8, D], BF16, kind="ExternalInput").ap()
    bq8_ext = nc.dram_tensor("bq8", [BL, D], F32, kind="ExternalInput").ap()
    bvt_ext = nc.dram_tensor("bvt", [128, 8], F32, kind="ExternalInput").ap()
    bo8_ext = nc.dram_tensor("bo8", [BL, D], F32, kind="ExternalInput").ap()
    y_ext = nc.dram_tensor("y", [BL, D], F32, kind="ExternalOutput").ap()

    with tile.TileContext(nc) as tc, ExitStack() as ctx:
        cpool = ctx.enter_context(tc.tile_pool(name="const", bufs=1))
        ident = cpool.tile([128, 128], F32)
        _make_identity(nc, ident[:])
        qt_sb = cpool.tile([128, 8, BL], BF16)
        nc.sync.dma_start(qt_sb[:], qt_ext[:])
        bvt = cpool.tile([128, 8], F32)
        nc.sync.dma_start(bvt[:], bvt_ext[:])

        # persistent across whole kernel
        R_all = cpool.tile([128, 8, HB], BF16)
        UT_all = cpool.tile([128, 8, H, BL], BF16)

        # ---------------- setup: qh^T, R ----------------
        with tc.tile_pool(name="wsetup", bufs=1) as wpool, \
             tc.tile_pool(name="spsum", bufs=1, space="PSUM") as spsum:
            bq8 = wpool.tile([BL, D], F32)
            nc.sync.dma_start(bq8[:], bq8_ext[:])
            # chunked weight loads spread over all 3 queues so the 4.2 MB
            # R-chain input lands in ~1/3 the time; R compute chases chunks
            QS0 = [nc.sync, nc.scalar, nc.gpsimd]
            wq_sb = wpool.tile([128, 8, D], BF16)
            wkt_sb = wpool.tile([128, 8, D], BF16)
            for cch in range(8):
                QS0[cch % 3].dma_start(wq_sb[:, cch, :], wq_ext[:, cch, :])
                QS0[(cch + 1) % 3].dma_start(wkt_sb[:, cch, :], wkt_ext[:, cch, :])

            # qh = q @ Wq + bq   [BL, D]
            qhp = spsum.tile([BL, D], F32, tag="qhp")
            for i in range(8):
                for n in range(2):
                    nc.tensor.matmul(qhp[:, n * 512:(n + 1) * 512],
                                     qt_sb[:, i, :],
                                     wq_sb[:, i, n * 512:(n + 1) * 512],
                                     start=(i == 0), stop=(i == 7))
            qh_sb = wpool.tile([BL, D], F32)
            nc.vector.tensor_add(qh_sb[:], qhp[:], bq8[:])
            qtp = spsum.tile([128, 8 * BL], F32, tag="qtp")
            for m in range(8):
                nc.tensor.transpose(qtp[:, m * BL:(m + 1) * BL],
                                    qh_sb[:, m * 128:(m + 1) * 128],
                                    ident[:BL, :BL])
            qhT_sb = wpool.tile([128, 8 * BL], F32)   # [p, m*BL + b]
            nc.vector.tensor_copy(qhT_sb[:], qtp[:])

            # Block-diagonal qh (bf16) for ALL batches:
            # qblk_c[p, b*16+h] = qh_b[c*128+p] if h == head(c*128+p) else 0
            qblk = [wpool.tile([128, HB], BF16, tag=f"qblk{c}", name=f"qblk{c}")
                    for c in range(8)]
            for c in range(8):
                nc.vector.memset(qblk[c][:], 0.0)
                lo = qblk[c][0:64, :].rearrange("p (b h) -> p b h", h=H)
                hi = qblk[c][64:128, :].rearrange("p (b h) -> p b h", h=H)
                nc.vector.tensor_copy(
                    lo[:, :, 2 * c:2 * c + 1],
                    qhT_sb[0:64, c * BL:(c + 1) * BL].unsqueeze(2))
                nc.vector.tensor_copy(
                    hi[:, :, 2 * c + 1:2 * c + 2],
                    qhT_sb[64:128, c * BL:(c + 1) * BL].unsqueeze(2))

            # RT[(b,h), d] = sum_c qblk_c^T @ WkT_c
            rtp = [spsum.tile([HB, 512], F32, tag=f"rtp{n}", name=f"rtp{n}")
                   for n in range(2)]
            for c in range(8):
                for n in range(2):
                    nc.tensor.matmul(rtp[n][:], qblk[c][:],
                                     wkt_sb[:, c, n * 512:(n + 1) * 512],
                                     start=(c == 0), stop=(c == 7))
            RT_sb = wpool.tile([HB, D], F32)
            for n in range(2):
                nc.vector.tensor_copy(RT_sb[:, n * 512:(n + 1) * 512], rtp[n][:])
            for j in range(8):
                rp = spsum.tile([128, HB], F32, tag="rp", name="rp")
                nc.tensor.transpose(rp[:], RT_sb[:, j * 128:(j + 1) * 128],
                                    ident[:HB, :HB])
                nc.vector.tensor_copy(R_all[:, j, :], rp[:])

        # ---------------- stream pools (reuse setup SBUF) ----------------
        tailw = ctx.enter_context(tc.tile_pool(name="tailw", bufs=1))
        wv_sb = tailw.tile([128, 8, D], BF16)
        wo_sb = tailw.tile([128, 8, D], BF16)
        bo8 = tailw.tile([BL, D], F32)

        stream_sbuf = ExitStack()
        epool = stream_sbuf.enter_context(tc.tile_pool(name="epool", bufs=2))
        etpool = stream_sbuf.enter_context(tc.tile_pool(name="etpool", bufs=2))
        upool = stream_sbuf.enter_context(tc.tile_pool(name="upool", bufs=2))
        ktpool = stream_sbuf.enter_context(tc.tile_pool(name="ktpool", bufs=3))
        vpool = stream_sbuf.enter_context(tc.tile_pool(name="vpool", bufs=3))

        stream_psum = ExitStack()
        scp = stream_psum.enter_context(tc.tile_pool(name="scp", bufs=2, space="PSUM"))
        upp = stream_psum.enter_context(tc.tile_pool(name="upp", bufs=1, space="PSUM"))
        tpp = stream_psum.enter_context(tc.tile_pool(name="tpp", bufs=1, space="PSUM"))
        tp2 = stream_psum.enter_context(tc.tile_pool(name="tp2", bufs=1, space="PSUM"))

        # rotate each batch's three pieces (kt, v-lower, v-upper) across the
        # three DMA queues so arrival order tracks consumption order and all
        # queues carry ~19-21 MB.  kt is issued 3 batches ahead (bufs=4, it
        # gates scores); v only 1 ahead so its buffer-wait (bufs=2) never
        # blocks a queue head.
        QS = [nc.sync, nc.scalar, nc.gpsimd]

        def load_kt(b):
            kt_t = ktpool.tile([128, 8, S], FP8, tag="kt", name="kt")
            QS[b % 3].dma_start(kt_t[:], kt_ext[b])
            return kt_t

        def load_v(b):
            vf = vpool.tile([128, SC, D], BF16, tag="vf", name="vf")
            half = SC // 2
            QS[(b + 1) % 3].dma_start(vf[:, :half, :], vt_ext[b, :, :half, :])
            QS[(b + 2) % 3].dma_start(vf[:, half:, :], vt_ext[b, :, half:, :])
            return vf

        kts = [load_kt(0)]
        vfs = [load_v(0)]
        if BL > 1:
            kts.append(load_kt(1))

        # ---------------- stream phase (software-pipelined) ----------------
        # PE issue order per iteration: scores(b) then U(b-1) — so a late
        # v(b-1) can never head-block scores(b) in the in-order PE queue,
        # and the PE gets 64 back-to-back big matmuls per iteration (stays
        # at full DVFS ramp).
        SGG = max(SG // 2, 1)       # [H, 1024] score blocks
        W2 = min(S, 1024)

        def scores_phase(b, kt_t):
            den4 = epool.tile([H, SGG], F32, tag="den4")
            scs = []
            for g in range(SGG):
                sc = scp.tile([H, W2], F32, tag="sc")
                for j in range(8):
                    for n in range(W2 // 512):
                        nc.tensor.matmul(
                            sc[:, n * 512:(n + 1) * 512],
                            R_all[:, j, b * H:(b + 1) * H],
                            kt_t[:, j, g * W2 + n * 512:g * W2 + (n + 1) * 512],
                            start=(j == 0), stop=(j == 7))
                scs.append(sc)
            return den4, scs

        def exp_trans_phase(b, den4, scs):
            Es = []
            for g, sc in enumerate(scs):
                E_g = epool.tile([H, W2], F32, tag="E")
                nc.scalar.activation(E_g[:], sc[:], ACTF.Exp, scale=0.125,
                                     accum_out=den4[:, g:g + 1])
                Es.append(E_g)
            sp = tpp.tile([128, SC * H], F32, tag="sp")
            for g, E_g in enumerate(Es):
                for i in range(W2 // 128):
                    t = g * (W2 // 128) + i
                    nc.tensor.transpose(sp[:, t * H:(t + 1) * H],
                                        E_g[:, i * 128:(i + 1) * 128],
                                        ident[:H, :H])
            ET = etpool.tile([128, SC, H], BF16, tag="ET")
            nc.vector.tensor_copy(
                ET[:], sp[:].rearrange("p (t h) -> p t h", t=SC))
            return ET

        def u_mm_phase(ET, vf):
            up = upp.tile([H, D], F32, tag="up")
            for cc in range(SC):
                for n in range(2):
                    nc.tensor.matmul(up[:, n * 512:(n + 1) * 512],
                                     ET[:, cc, :],
                                     vf[:, cc, n * 512:(n + 1) * 512],
                                     start=(cc == 0), stop=(cc == SC - 1))
            return up

        def u_finish_phase(b, den4, up):
            den = epool.tile([H, 1], F32, tag="den")
            nc.vector.tensor_reduce(den[:], den4[:], axis=AX.X, op=ALU.add)
            rden = epool.tile([H, 1], F32, tag="rden")
            nc.vector.reciprocal(rden[:], den[:])
            U_sb = upool.tile([H, D], F32, tag="U")
            nc.vector.tensor_scalar_mul(U_sb[:], up[:], rden[:])
            sp2 = tp2.tile([128, 8 * H], F32, tag="sp2")
            for jc in range(8):
                nc.tensor.transpose(sp2[:, jc * H:(jc + 1) * H],
                                    U_sb[:, jc * 128:(jc + 1) * 128],
                                    ident[:H, :H])
            nc.vector.tensor_copy(
                UT_all[:, :, :, b],
                sp2[:].rearrange("p (j h) -> p j h", j=8))

        prev = None     # (b, den4, ET)
        for b in range(BL):
            if b + 1 < BL:
                vfs.append(load_v(b + 1))
            if b + 2 < BL:
                kts.append(load_kt(b + 2))
            if b == min(4, BL - 1):
                nc.sync.dma_start(wv_sb[:], wv_ext[:])
                nc.scalar.dma_start(wo_sb[:], wo_ext[:])
                nc.sync.dma_start(bo8[:], bo8_ext[:])


            den4, scs = scores_phase(b, kts[b])
            if prev is not None:
                pb, pden4, pET = prev
                pup = u_mm_phase(pET, vfs[pb])
            ET = exp_trans_phase(b, den4, scs)
            if prev is not None:
                u_finish_phase(pb, pden4, pup)
            prev = (b, den4, ET)

        pb, pden4, pET = prev
        pup = u_mm_phase(pET, vfs[pb])
        u_finish_phase(pb, pden4, pup)

        # ---------------- tail: out-projection ----------------
        # ocT[col, (h,b)] = sum_d Wv[d, col] U[b,h,d] for ALL (col, h) pairs;
        # valid block-diagonal (h == col//64) extracted during the PSUM->SBUF
        # copy, directly in transposed (OCT) layout for y = OC@Wo.
        stream_psum.close()
        stream_sbuf.close()
        with tc.tile_pool(name="fin", bufs=1) as fpool, \
             tc.tile_pool(name="fpsum", bufs=2, space="PSUM") as fpsum:
            # pipelined: oct(c+1) matmuls overlap the DVE extraction of
            # oct(c); ypp(c) follows one chunk behind so the PE never waits
            OCT = fpool.tile([128, 8, BL], BF16)
            ypp = fpsum.tile([BL, D], F32, tag="yp")

            def oct_chunk(c):
                oct_ps = fpsum.tile([128, HB], F32, tag="oct", name="oct")
                for jc in range(8):
                    nc.tensor.matmul(oct_ps[:],
                                     wv_sb[:, jc, c * 128:(c + 1) * 128],
                                     UT_all[:, jc, :, :],
                                     start=(jc == 0), stop=(jc == 7))
                for half in range(2):
                    h = 2 * c + half
                    sl = slice(half * 64, (half + 1) * 64)
                    nc.vector.tensor_scalar_add(
                        OCT[sl, c, :], oct_ps[sl, h * BL:(h + 1) * BL],
                        bvt[sl, c:c + 1])

            def ypp_chunk(c):
                for n in range(2):
                    nc.tensor.matmul(ypp[:, n * 512:(n + 1) * 512],
                                     OCT[:, c, :],
                                     wo_sb[:, c, n * 512:(n + 1) * 512],
                                     start=(c == 0), stop=(c == 7))

            oct_chunk(0)
            for c in range(1, 8):
                oct_chunk(c)
                ypp_chunk(c - 1)
            ypp_chunk(7)
            ytmp = fpool.tile([BL, D], F32)
            nc.vector.tensor_add(ytmp[:], ypp[:], bo8[:])
            y_sb = fpool.tile([BL, D], F32)
            nc.vector.tensor_scalar_max(y_sb[:], ytmp[:], 0.0)
            nc.sync.dma_start(y_ext[:], y_sb[:])

    nc.compile()
    return nc


_built = {}


def _get_nc(BL, S):
    key = (BL, S)
    if key not in _built:
        _built[key] = build(BL, S)
    return _built[key]


def kernel(q, k, v, Wq, bq, Wk, bk, Wv, bv, Wo, bo, _trace=False):
    q = np.asarray(q, dtype=np.float32)
    k = np.asarray(k, dtype=np.float32)
    v = np.asarray(v, dtype=np.float32)
    B, S = k.shape[0], k.shape[1]
    BL = B // N_CORES
    SC = S // 128
    nc = _get_nc(BL, S)

    # host-side staging: dtype + layout only (all model math is on-device)
    kt_all = k.astype(NP_FP8).reshape(B, S, 8, 128).transpose(0, 3, 2, 1)
    vt_all = v.astype(NP_BF16).reshape(B, SC, 128, D).transpose(0, 2, 1, 3)
    qt_all = q.reshape(B, 8, 128).transpose(2, 1, 0).astype(NP_BF16)  # [128,8,B]

    shared = {
        "wq": np.ascontiguousarray(
            Wq.astype(NP_BF16).reshape(8, 128, D).transpose(1, 0, 2)),
        "wkt": np.ascontiguousarray(
            np.ascontiguousarray(Wk.T).astype(NP_BF16)
            .reshape(8, 128, D).transpose(1, 0, 2)),
        "wv": np.ascontiguousarray(
            Wv.astype(NP_BF16).reshape(8, 128, D).transpose(1, 0, 2)),
        "wo": np.ascontiguousarray(
            Wo.astype(NP_BF16).reshape(8, 128, D).transpose(1, 0, 2)),
        "bvt": np.ascontiguousarray(
            np.asarray(bv, dtype=np.float32).reshape(8, 128).T),
        "bo8": np.ascontiguousarray(np.broadcast_to(
            np.asarray(bo, dtype=np.float32), (BL, D))),
        "bq8": np.ascontiguousarray(np.broadcast_to(
            np.asarray(bq, dtype=np.float32), (BL, D))),
    }
    in_maps = []
    for c in range(N_CORES):
        sl = slice(c * BL, (c + 1) * BL)
        in_maps.append({
            "kt": np.ascontiguousarray(kt_all[sl]),
            "vt": np.ascontiguousarray(vt_all[sl]),
            "qt": np.ascontiguousarray(qt_all[:, :, sl]),
            **shared,
        })
    res = run_bass_kernel_spmd(nc, in_maps, list(range(N_CORES)), trace=_trace)
    out = np.concatenate([res.results[c]["y"] for c in range(N_CORES)], axis=0)
    if _trace:
        kernel._last_exec_time_ns = res.exec_time_ns
        kernel._last_profile = res.profile_json
    return out


# revision 54
# speedup vs baseline: 1.0296x; 1.0133x over previous
"""Trainium2 Bass kernel for nn_MultiHeadAttention (decode-style, q_len=1).

Data-parallel over batch: 64 batches -> 8 cores x 8 batches.

Algebraic restructuring (exact, exploits q_len == 1):
  scores[b,h,s] = k[b,s,:] . R_b[:,h] + const(b,h)   # const drops in softmax
     where R_b[d,h] = sum_{d'} Wk[d, h*64+d'] qh[b, h*64+d']
  out_concat[b,hd] = (sum_s p[b,h,s] v[b,s,:]) @ Wv[:,hd] + bv[hd]
so the big K/V projections are never computed; k and v are contracted
directly and the kernel is HBM-bound on streaming k,v.

Precision/layout staging (host side, per-core):
  k  -> fp8 e3m4, pre-transposed [128(d%), 8(d/128), S]   (16 MiB/core)
  v  -> bf16,     chunked       [128(s%), S/128, 1024]    (32 MiB/core)
  Wq/Wk^T/Wv/Wo -> bf16 pre-transposed SBUF layouts       ( 8 MiB/core)
Total ~56 MiB/core HBM reads (vs 144 f32), no on-device transposes or
casts of the streams.  Numpy-simulated rel err 1.41e-2 (< 2e-2 gate);
the e3m4 path keeps 4 mantissa bits and max 15.5 (|k| <= ~5.5).

Schedule: THREE DMA queues (sync=SP HWDGE, scalar=Act HWDGE,
gpsimd=SWDGE) stream in parallel; each batch's three pieces (kt, v-lo,
v-hi) rotate across the queues so arrival order tracks consumption
order and all queues carry ~19-21 MB; Wq/Wk^T head the queues as 2KB
column chunks the R-chain chases; Wv/Wo slot in mid-stream.  The batch
loop is software-pipelined for the in-order PE queue: scores(b) [32
matmuls, bf16 R x fp8 k] issue before U(b-1) [32 matmuls, bf16 ET x
bf16 v], so a late v can never head-block the next batch's scores, and
exp(b) (scalar engine, fused den accumulation) overlaps U(b-1).  The
out-projection computes all (h,b) x Wv products in 8 wide matmuls and
extracts the valid block-diagonal in transposed (OCT) layout during
the PSUM->SBUF copy (fused with bv), then chases y = relu(OC@Wo + bo)
per column chunk, one chunk behind the extraction.
"""

import numpy as np
import ml_dtypes
from contextlib import ExitStack

import concourse.bass as bass
import concourse.tile as tile
from concourse import bacc, mybir
from concourse.bass_utils import run_bass_kernel_spmd

try:
    import axon_profile_shim
    axon_profile_shim.install()
except Exception:
    pass

N_CORES = 8
D = 1024
H = 16
DK = 64
F32 = mybir.dt.float32
BF16 = mybir.dt.bfloat16
FP8 = mybir.dt.float8e3
AX = mybir.AxisListType
ALU = mybir.AluOpType
ACTF = mybir.ActivationFunctionType

NP_BF16 = ml_dtypes.bfloat16
NP_FP8 = ml_dtypes.float8_e3m4


def _make_identity(nc, ap):
    nc.gpsimd.memset(ap, 0.0)
    nc.gpsimd.affine_select(
        out=ap, in_=ap, compare_op=ALU.not_equal, fill=1.0,
        base=0, pattern=[[-1, ap.shape[0]]], channel_multiplier=1,
    )


def build(BL=8, S=2048, n_cores=N_CORES):
    """Build + compile the per-core program. BL = local batches, S = seq len."""
    SC = S // 128           # 128-row s-chunks
    SG = S // 512           # 512-col score blocks
    HB = H * BL
    nc = bacc.Bacc("TRN2", target_bir_lowering=False, debug=False,
                   num_devices=n_cores)

    kt_ext = nc.dram_tensor("kt", [BL, 128, 8, S], FP8, kind="ExternalInput").ap()
    vt_ext = nc.dram_tensor("vt", [BL, 128, SC, D], BF16, kind="ExternalInput").ap()
    qt_ext = nc.dram_tensor("qt", [128, 8, BL], BF16, kind="ExternalInput").ap()
    wq_ext = nc.dram_tensor("wq", [128, 8, D], BF16, kind="ExternalInput").ap()
    wkt_ext = nc.dram_tensor("wkt", [128, 8, D], BF16, kind="ExternalInput").ap()
    wv_ext = nc.dram_tensor("wv", [128, 8, D], BF16, kind="ExternalInput").ap()
    wo_ext = nc.dram_tensor("wo", [128, 8, D], BF16, kind="ExternalInput").ap()
    bq8_ext = nc.dram_tensor("bq8", [BL, D], F32, kind="ExternalInput").ap()
    bvt_ext = nc.dram_tensor("bvt", [128, 8], F32, kind="ExternalInput").ap()
    bo8_ext = nc.dram_tensor("bo8", [BL, D], F32, kind="ExternalInput").ap()
    y_ext = nc.dram_tensor("y", [BL, D], F32, kind="ExternalOutput").ap()

    with tile.TileContext(nc) as tc, ExitStack() as ctx:
        cpool = ctx.enter_context(tc.tile_pool(name="const", bufs=1))
        ident = cpool.tile([128, 128], F32)
        _make_identity(nc, ident[:])
        qt_sb = cpool.tile([128, 8, BL], BF16)
        nc.sync.dma_start(qt_sb[:], qt_ext[:])
        bvt = cpool.tile([128, 8], F32)
        nc.sync.dma_start(bvt[:], bvt_ext[:])

        # persistent across whole kernel
        R_all = cpool.tile([128, 8, HB], BF16)
        UT_all = cpool.tile([128, 8, H, BL], BF16)

        # ---------------- setup: qh^T, R ----------------
        with tc.tile_pool(name="wsetup", bufs=1) as wpool, \
             tc.tile_pool(name="spsum", bufs=1, space="PSUM") as spsum:
            bq8 = wpool.tile([BL, D], F32)
            nc.sync.dma_start(bq8[:], bq8_ext[:])
            # chunked weight loads spread over all 3 queues so the 4.2 MB
            # R-chain input lands in ~1/3 the time; R compute chases chunks
            QS0 = [nc.sync, nc.scalar, nc.gpsimd]
            wq_sb = wpool.tile([128, 8, D], BF16)
            wkt_sb = wpool.tile([128, 8, D], BF16)
            for cch in range(8):
                QS0[cch % 3].dma_start(wq_sb[:, cch, :], wq_ext[:, cch, :])
                QS0[(cch + 1) % 3].dma_start(wkt_sb[:, cch, :], wkt_ext[:, cch, :])

            # qh = q @ Wq + bq   [BL, D]
            qhp = spsum.tile([BL, D], F32, tag="qhp")
            for i in range(8):
                for n in range(2):
                    nc.tensor.matmul(qhp[:, n * 512:(n + 1) * 512],
                                     qt_sb[:, i, :],
                                     wq_sb[:, i, n * 512:(n + 1) * 512],
                                     start=(i == 0), stop=(i == 7))
            qh_sb = wpool.tile([BL, D], F32)
            nc.vector.tensor_add(qh_sb[:], qhp[:], bq8[:])
            qtp = spsum.tile([128, 8 * BL], F32, tag="qtp")
            for m in range(8):
                nc.tensor.transpose(qtp[:, m * BL:(m + 1) * BL],
                                    qh_sb[:, m * 128:(m + 1) * 128],
                                    ident[:BL, :BL])
            qhT_sb = wpool.tile([128, 8 * BL], F32)   # [p, m*BL + b]
            nc.vector.tensor_copy(qhT_sb[:], qtp[:])

            # Block-diagonal qh (bf16) for ALL batches:
            # qblk_c[p, b*16+h] = qh_b[c*128+p] if h == head(c*128+p) else 0
            qblk = [wpool.tile([128, HB], BF16, tag=f"qblk{c}", name=f"qblk{c}")
                    for c in range(8)]
            for c in range(8):
                nc.vector.memset(qblk[c][:], 0.0)
                lo = qblk[c][0:64, :].rearrange("p (b h) -> p b h", h=H)
                hi = qblk[c][64:128, :].rearrange("p (b h) -> p b h", h=H)
                nc.vector.tensor_copy(
                    lo[:, :, 2 * c:2 * c + 1],
                    qhT_sb[0:64, c * BL:(c + 1) * BL].unsqueeze(2))
                nc.vector.tensor_copy(
                    hi[:, :, 2 * c + 1:2 * c + 2],
                    qhT_sb[64:128, c * BL:(c + 1) * BL].unsqueeze(2))

            # RT[(b,h), d] = sum_c qblk_c^T @ WkT_c
            rtp = [spsum.tile([HB, 512], F32, tag=f"rtp{n}", name=f"rtp{n}")
                   for n in range(2)]
            for c in range(8):
                for n in range(2):
                    nc.tensor.matmul(rtp[n][:], qblk[c][:],
                                     wkt_sb[:, c, n * 512:(n + 1) * 512],
                                     start=(c == 0), stop=(c == 7))
            RT_sb = wpool.tile([HB, D], F32)
            for n in range(2):
                nc.vector.tensor_copy(RT_sb[:, n * 512:(n + 1) * 512], rtp[n][:])
            for j in range(8):
                rp = spsum.tile([128, HB], F32, tag="rp", name="rp")
                nc.tensor.transpose(rp[:], RT_sb[:, j * 128:(j + 1) * 128],
                                    ident[:HB, :HB])
                nc.vector.tensor_copy(R_all[:, j, :], rp[:])

        # ---------------- stream pools (reuse setup SBUF) ----------------
        tailw = ctx.enter_context(tc.tile_pool(name="tailw", bufs=1))
        wv_sb = tailw.tile([128, 8, D], BF16)
        wo_sb = tailw.tile([128, 8, D], BF16)
        bo8 = tailw.tile([BL, D], F32)

        stream_sbuf = ExitStack()
        epool = stream_sbuf.enter_context(tc.tile_pool(name="epool", bufs=2))
        etpool = stream_sbuf.enter_context(tc.tile_pool(name="etpool", bufs=2))
        upool = stream_sbuf.enter_context(tc.tile_pool(name="upool", bufs=2))
        ktpool = stream_sbuf.enter_context(tc.tile_pool(name="ktpool", bufs=3))
        vpool = stream_sbuf.enter_context(tc.tile_pool(name="vpool", bufs=3))

        stream_psum = ExitStack()
        scp = stream_psum.enter_context(tc.tile_pool(name="scp", bufs=2, space="PSUM"))
        upp = stream_psum.enter_context(tc.tile_pool(name="upp", bufs=1, space="PSUM"))
        tpp = stream_psum.enter_context(tc.tile_pool(name="tpp", bufs=1, space="PSUM"))
        tp2 = stream_psum.enter_context(tc.tile_pool(name="tp2", bufs=1, space="PSUM"))

        # rotate each batch's three pieces (kt, v-lower, v-upper) across the
        # three DMA queues so arrival order tracks consumption order and all
        # queues carry ~19-21 MB.  kt is issued 3 batches ahead (bufs=4, it
        # gates scores); v only 1 ahead so its buffer-wait (bufs=2) never
        # blocks a queue head.
        QS = [nc.sync, nc.scalar, nc.gpsimd]

        def load_kt(b):
            kt_t = ktpool.tile([128, 8, S], FP8, tag="kt", name="kt")
            QS[b % 3].dma_start(kt_t[:], kt_ext[b])
            return kt_t

        def load_v(b):
            vf = vpool.tile([128, SC, D], BF16, tag="vf", name="vf")
            half = SC // 2
            QS[(b + 1) % 3].dma_start(vf[:, :half, :], vt_ext[b, :, :half, :])
            QS[(b + 2) % 3].dma_start(vf[:, half:, :], vt_ext[b, :, half:, :])
            return vf

        kts = [load_kt(0)]
        vfs = [load_v(0)]
        if BL > 1:
            kts.append(load_kt(1))

        # ---------------- stream phase (software-pipelined) ----------------
        # PE issue order per iteration: scores(b) then U(b-1) — so a late
        # v(b-1) can never head-block scores(b) in the in-order PE queue,
        # and the PE gets 64 back-to-back big matmuls per iteration (stays
        # at full DVFS ramp).
        SGG = max(SG // 2, 1)       # [H, 1024] score blocks
        W2 = min(S, 1024)

        def scores_phase(b, kt_t):
            den4 = epool.tile([H, SGG], F32, tag="den4")
            scs = []
            for g in range(SGG):
                sc = scp.tile([H, W2], F32, tag="sc")
                for j in range(8):
                    for n in range(W2 // 512):
                        nc.tensor.matmul(
                            sc[:, n * 512:(n + 1) * 512],
                            R_all[:, j, b * H:(b + 1) * H],
                            kt_t[:, j, g * W2 + n * 512:g * W2 + (n + 1) * 512],
                            start=(j == 0), stop=(j == 7))
                scs.append(sc)
            return den4, scs

        def exp_trans_phase(b, den4, scs):
            Es = []
            for g, sc in enumerate(scs):
                E_g = epool.tile([H, W2], F32, tag="E")
                nc.scalar.activation(E_g[:], sc[:], ACTF.Exp, scale=0.125,
                                     accum_out=den4[:, g:g + 1])
                Es.append(E_g)
            sp = tpp.tile([128, SC * H], F32, tag="sp")
            for g, E_g in enumerate(Es):
                for i in range(W2 // 128):
                    t = g * (W2 // 128) + i
                    nc.tensor.transpose(sp[:, t * H:(t + 1) * H],
                                        E_g[:, i * 128:(i + 1) * 128],
                                        ident[:H, :H])
            ET = etpool.tile([128, SC, H], BF16, tag="ET")
            nc.vector.tensor_copy(
                ET[:], sp[:].rearrange("p (t h) -> p t h", t=SC))
            return ET

        def u_mm_phase(ET, vf):
            up = upp.tile([H, D], F32, tag="up")
            for cc in range(SC):
                for n in range(2):
                    nc.tensor.matmul(up[:, n * 512:(n + 1) * 512],
                                     ET[:, cc, :],
                                     vf[:, cc, n * 512:(n + 1) * 512],
                                     start=(cc == 0), stop=(cc == SC - 1))
            return up

        def u_finish_phase(b, den4, up):
            den = epool.tile([H, 1], F32, tag="den")
            nc.vector.tensor_reduce(den[:], den4[:], axis=AX.X, op=ALU.add)
            rden = epool.tile([H, 1], F32, tag="rden")
            nc.vector.reciprocal(rden[:], den[:])
            U_sb = upool.tile([H, D], F32, tag="U")
            nc.vector.tensor_scalar_mul(U_sb[:], up[:], rden[:])
            sp2 = tp2.tile([128, 8 * H], F32, tag="sp2")
            for jc in range(8):
                nc.tensor.transpose(sp2[:, jc * H:(jc + 1) * H],
                                    U_sb[:, jc * 128:(jc + 1) * 128],
                                    ident[:H, :H])
            nc.vector.tensor_copy(
                UT_all[:, :, :, b],
                sp2[:].rearrange("p (j h) -> p j h", j=8))

        prev = None     # (b, den4, ET)
        for b in range(BL):
            if b + 1 < BL:
                vfs.append(load_v(b + 1))
            if b + 2 < BL:
                kts.append(load_kt(b + 2))
            if b == min(4, BL - 1):
                nc.sync.dma_start(wv_sb[:], wv_ext[:])
                nc.scalar.dma_start(wo_sb[:], wo_ext[:])
                nc.sync.dma_start(bo8[:], bo8_ext[:])


            den4, scs = scores_phase(b, kts[b])
            if prev is not None:
                pb, pden4, pET = prev
                pup = u_mm_phase(pET, vfs[pb])
            ET = exp_trans_phase(b, den4, scs)
            if prev is not None:
                u_finish_phase(pb, pden4, pup)
            prev = (b, den4, ET)

        pb, pden4, pET = prev
        pup = u_mm_phase(pET, vfs[pb])
        u_finish_phase(pb, pden4, pup)

        # ---------------- tail: out-projection ----------------
        # ocT[col, (h,b)] = sum_d Wv[d, col] U[b,h,d] for ALL (col, h) pairs;
        # valid block-diagonal (h == col//64) extracted during the PSUM->SBUF
        # copy, directly in transposed (OCT) layout for y = OC@Wo.
        stream_psum.close()
        stream_sbuf.close()
        with tc.tile_pool(name="fin", bufs=1) as fpool, \
             tc.tile_pool(name="fpsum", bufs=2, space="PSUM") as fpsum:
            # pipelined: oct(c+1) matmuls overlap the DVE extraction of
            # oct(c); ypp(c) follows one chunk behind so the PE never waits
            OCT = fpool.tile([128, 8, BL], BF16)
            ypp = fpsum.tile([BL, D], F32, tag="yp")

            def oct_chunk(c):
                oct_ps = fpsum.tile([128, HB], F32, tag="oct", name="oct")
                for jc in range(8):
                    nc.tensor.matmul(oct_ps[:],
                                     wv_sb[:, jc, c * 128:(c + 1) * 128],
                                     UT_all[:, jc, :, :],
                                     start=(jc == 0), stop=(jc == 7))
                for half in range(2):
                    h = 2 * c + half
                    sl = slice(half * 64, (half + 1) * 64)
                    nc.vector.tensor_scalar_add(
                        OCT[sl, c, :], oct_ps[sl, h * BL:(h + 1) * BL],
                        bvt[sl, c:c + 1])

            def ypp_chunk(c):
                for n in range(2):
                    nc.tensor.matmul(ypp[:, n * 512:(n + 1) * 512],
                                     OCT[:, c, :],
                                     wo_sb[:, c, n * 512:(n + 1) * 512],
                                     start=(c == 0), stop=(c == 7))

            oct_chunk(0)
            for c in range(1, 8):
                oct_chunk(c)
                ypp_chunk(c - 1)
            ypp_chunk(7)
            ytmp = fpool.tile([BL, D], F32)
            nc.vector.tensor_add(ytmp[:], ypp[:], bo8[:])
            y_sb = fpool.tile([BL, D], F32)
            nc.vector.tensor_scalar_max(y_sb[:], ytmp[:], 0.0)
            nc.sync.dma_start(y_ext[:], y_sb[:])

    nc.compile()
    return nc


_built = {}


def _get_nc(BL, S):
    key = (BL, S)
    if key not in _built:
        _built[key] = build(BL, S)
    return _built[key]


def kernel(q, k, v, Wq, bq, Wk, bk, Wv, bv, Wo, bo, _trace=False):
    q = np.asarray(q, dtype=np.float32)
    k = np.asarray(k, dtype=np.float32)
    v = np.asarray(v, dtype=np.float32)
    B, S = k.shape[0], k.shape[1]
    BL = B // N_CORES
    SC = S // 128
    nc = _get_nc(BL, S)

    # host-side staging: dtype + layout only (all model math is on-device)
    kt_all = k.astype(NP_FP8).reshape(B, S, 8, 128).transpose(0, 3, 2, 1)
    vt_all = v.astype(NP_BF16).reshape(B, SC, 128, D).transpose(0, 2, 1, 3)
    qt_all = q.reshape(B, 8, 128).transpose(2, 1, 0).astype(NP_BF16)  # [128,8,B]

    shared = {
        "wq": np.ascontiguousarray(
            Wq.astype(NP_BF16).reshape(8, 128, D).transpose(1, 0, 2)),
        "wkt": np.ascontiguousarray(
            np.ascontiguousarray(Wk.T).astype(NP_BF16)
            .reshape(8, 128, D).transpose(1, 0, 2)),
        "wv": np.ascontiguousarray(
            Wv.astype(NP_BF16).reshape(8, 128, D).transpose(1, 0, 2)),
        "wo": np.ascontiguousarray(
            Wo.astype(NP_BF16).reshape(8, 128, D).transpose(1, 0, 2)),
        "bvt": np.ascontiguousarray(
            np.asarray(bv, dtype=np.float32).reshape(8, 128).T),
        "bo8": np.ascontiguousarray(np.broadcast_to(
            np.asarray(bo, dtype=np.float32), (BL, D))),
        "bq8": np.ascontiguousarray(np.broadcast_to(
            np.asarray(bq, dtype=np.float32), (BL, D))),
    }
    in_maps = []
    for c in range(N_CORES):
        sl = slice(c * BL, (c + 1) * BL)
        in_maps.append({
            "kt": np.ascontiguousarray(kt_all[sl]),
            "vt": np.ascontiguousarray(vt_all[sl]),
            "qt": np.ascontiguousarray(qt_all[:, :, sl]),
            **shared,
        })
    res = run_bass_kernel_spmd(nc, in_maps, list(range(N_CORES)), trace=_trace)
    out = np.concatenate([res.results[c]["y"] for c in range(N_CORES)], axis=0)
    if _trace:
        kernel._last_exec_time_ns = res.exec_time_ns
        kernel._last_profile = res.profile_json
    return out
